# revision 1
# baseline (speedup 1.0000x reference)
"""AdderNet depthwise 3x3 L1-distance conv for Trainium2, 8-core data parallel.

out[b,c,h,w] = -sum_{i,j in 3x3} |x_pad[b,c,h+i,w+j] - W[c,0,i,j]|

Strategy (per core, 4 images of the batch = 16 (b,c) planes):
- Host zero-pads each [512,512] plane to [514,514]; pads ARE semantic
  (reference pads with zeros inside the |.| sum).
- Per plane: 4 row-blocks of 128 rows live on the 128 SBUF partitions
  (block index on the free dim). W-shifts (j) are free-dim offsets.
- 9 abs-diff tap planes |x - w[c,i,j]| in bf16: 5 on ScalarE via
  activation(Abs, bias=-w), 4 on VectorE via a registered custom DVE op
  (relu(x-w) + relu(w-x); tensor_scalar has no legal fused abs on TRN2).
- H-shifts (i) + the 9-tap sum run on TensorE: 3 shifted-identity bf16
  matrices as stationary operands, 9 accumulating matmuls per PSUM bank.
- PSUM evacuation (with the output negation via scale=-1) is split
  between VectorE (2 blocks) and ScalarE (2 blocks).
- Row-block seams (2 rows per 128-block) are patched by 2 small fixup
  tiles that pack 4-row bands around every seam for 8 images each
  (partition layout q = 32*band + 4*image + row).

Measured on 8 axon trn2 cores: HW exec ~230 us/core, rel err 2.2e-3
(bf16 tap rounding; PSUM accumulation is f32). Engine balance per
supertile: PE ~8.3us (36 MM + LDW), ACT ~10.5us, DVE ~9.9us.
"""

import numpy as np
import ml_dtypes

B, C, H, W = 32, 4, 512, 512
N_CORES = 8
B_LOC = B // N_CORES          # 4 images per core
N_IMG = B_LOC * C             # 16 (b,c) planes per core
HP, WP = H + 2, W + 2         # 514, 514
NBLK = 4                      # row blocks of 128 per plane (rows 0..511 of padded)
P = 128

# tap t = 3*i + j ; taps computed on ScalarE (rest on VectorE custom absdiff)
ACT_TAPS = (0, 2, 4, 6, 8)
# psum blocks evacuated by VectorE (rest by ScalarE)
DVE_EVAC_BLOCKS = 2

_PROGRAM_CACHE = {}


def _register_absdiff():
    """Register a custom DVE op: out = |in0 - s0| = relu(in0-s0) + relu(s0-in0).
    tensor_scalar has no legal fused abs on TRN2 (abs_max fails the ISA check,
    arith+bitwise ops can't mix), so this 1-instruction DVE op is the cheapest
    legal per-tap absdiff."""
    from concourse import dve_ops
    from concourse.dve_spec import Spec, Src0, C0, relu, lower
    from concourse.dve_uop import DveOpSpec

    for o in dve_ops.OPS:
        if o.name == "ABS_DIFF_ANT":
            return o
    def _ref(in0, in1, s0, s1, imm2):
        s = np.asarray(s0)
        if s.ndim and in0.ndim > s.ndim:  # [P,1] scalar vs [P,S,N] tensor
            s = s.reshape(s.shape[0], *([1] * (in0.ndim - 1)))
        return np.abs(in0.astype(np.float32) - s).astype(np.float32)

    spec = Spec(
        body=relu(Src0 - C0) + relu(C0 - Src0),
        reference=_ref,
    )
    shas = {
        ver: DveOpSpec(name="ABS_DIFF_ANT", uops=lower(spec, ver=ver)).sha(ver)
        for ver in ("v3", "v4")
    }
    op = dve_ops.DveOp("ABS_DIFF_ANT", spec, subdim=False, uops_sha=shas)
    dve_ops.OPS.append(op)
    dve_ops.CUSTOM_DVE_SPECS[op.name] = spec
    dve_ops._SUB_OPCODE_FOR_NAME[op.name] = dve_ops._CUSTOM_DVE_ROW_BASE + len(dve_ops.OPS) - 1
    return op


def _patch_ldw_opt():
    """walrus dedups back-to-back LDWEIGHTS of the same stationary tensor
    only with --enable-ldw-opt; concourse hardcodes it off. Our inner loop
    issues 12 consecutive matmuls per stationary shift matrix, and the
    per-matmul reload serializes PE fill/drain (379ns/MM vs 216 target)."""
    import concourse.bass_utils as bu

    if getattr(bu, "_ldw_patched", False):
        return
    orig = bu.run_command

    def patched(argv, **kw):
        argv = [
            a
            for a in argv
        ]
        return orig(argv, **kw)

    bu.run_command = patched
    bu._ldw_patched = True


def _build_program():
    import concourse.mybir as mybir
    import concourse.tile as tile
    from concourse import bacc

    _patch_ldw_opt()

    f32 = mybir.dt.float32
    bf16 = mybir.dt.bfloat16
    absdiff = _register_absdiff()
    nc = bacc.Bacc("TRN2", target_bir_lowering=False)

    xpad = nc.declare_dram_parameter("xpad", [N_IMG, HP, WP], f32, isOutput=False)
    smat = nc.declare_dram_parameter("smat", [3, P, P], bf16, isOutput=False)
    # bias[:, st*18 + t] : +w for DVE taps (cols 0-8), -w for ACT taps (9-17);
    # supertiles 0-15 then fixup tiles 16-17
    bias = nc.declare_dram_parameter("bias", [P, (N_IMG + 2) * 18], f32, isOutput=False)
    outp = nc.declare_dram_parameter("outp", [N_IMG, HP, W], f32, isOutput=True)

    with tile.TileContext(nc) as tc:
        with (
            tc.tile_pool(name="const", bufs=1) as cpool,
            tc.tile_pool(name="xp", bufs=3) as xpool,
            tc.tile_pool(name="dp", bufs=16) as dpool,
            tc.tile_pool(name="op", bufs=4) as opool,
            tc.tile_pool(name="ps", bufs=2, space="PSUM") as ppool,
        ):
            s_t = cpool.tile([P, 3, P], bf16, tag="s")
            nc.sync.dma_start(out=s_t, in_=smat[:].rearrange("s k p -> k s p"))
            b_all = cpool.tile([P, (N_IMG + 2) * 18], f32, tag="ball")
            nc.sync.dma_start(out=b_all, in_=bias[:])

            # Warmup activations with minimal deps so walrus attaches the ACT
            # table-load pseudo-instruction here (a loaded instruction with 2
            # DMA waits + table load exceeds the sync-wait slots).
            warm = cpool.tile([P, 2], f32, tag="warm")
            nc.vector.memset(warm, 0.0)
            nc.scalar.activation(
                out=warm[:, 0:1],
                in_=warm[:, 1:2],
                func=mybir.ActivationFunctionType.Abs,
                bias=0.0,
                scale=1.0,
            )
            nc.scalar.activation(
                out=warm[:, 1:2],
                in_=warm[:, 0:1],
                func=mybir.ActivationFunctionType.Copy,
                scale=-1.0,
            )

            def _evac(ps, st):
                o_t = opool.tile([P, NBLK, W], f32, tag="o")
                eb = DVE_EVAC_BLOCKS
                nc.vector.tensor_scalar(
                    out=o_t[:, :eb],
                    in0=ps[:, :eb],
                    scalar1=-1.0,
                    scalar2=None,
                    op0=mybir.AluOpType.mult,
                )
                nc.scalar.activation(
                    out=o_t[:, eb:],
                    in_=ps[:, eb:],
                    func=mybir.ActivationFunctionType.Copy,
                    scale=-1.0,
                )
                nc.sync.dma_start(
                    out=outp[st, 0:512, :].rearrange("(b q) w -> q b w", q=P)[1:127],
                    in_=o_t[1:127],
                )

            def _fixup(fi):
                g0 = fi * 8
                xf = xpool.tile([P, WP], f32, tag="xf")
                for band in range(4):
                    nc.sync.dma_start(
                        out=xf[32 * band : 32 * (band + 1)],
                        in_=xpad[g0 : g0 + 8, 126 + 128 * band : 130 + 128 * band, :],
                    )
                bofs = (N_IMG + fi) * 18
                df_tiles = []
                for t in range(9):
                    i, j = divmod(t, 3)
                    d = dpool.tile([P, W], bf16, tag="d")
                    src = xf[:, j : j + W]
                    if t in ACT_TAPS:
                        nc.scalar.activation(
                            out=d,
                            in_=src,
                            func=mybir.ActivationFunctionType.Abs,
                            bias=b_all[:, bofs + 9 + t : bofs + 10 + t],
                            scale=1.0,
                        )
                    else:
                        nc.vector._custom_dve(
                            absdiff,
                            out=d,
                            in0=src,
                            s0=b_all[:, bofs + t : bofs + t + 1],
                        )
                    df_tiles.append(d)

                pf = ppool.tile([P, W], mybir.dt.float32, tag="ps")
                for i in range(3):
                    t0 = 3 * i
                    for j in range(3):
                        nc.tensor.matmul(
                            pf,
                            lhsT=s_t[:, i, :],
                            rhs=df_tiles[t0 + j],
                            start=(t0 + j == 0),
                            stop=(t0 + j == 8),
                        )

                of = opool.tile([P, W], f32, tag="o")
                nc.scalar.activation(
                    out=of,
                    in_=pf,
                    func=mybir.ActivationFunctionType.Copy,
                    scale=-1.0,
                )
                for band in range(4):
                    for g in range(8):
                        lo = 32 * band + 4 * g
                        nc.sync.dma_start(
                            out=outp[g0 + g, 127 + 128 * band : 129 + 128 * band, :],
                            in_=of[lo + 1 : lo + 3],
                        )

            pending = None
            # ---- main supertiles: one per (b,c) plane ----
            for st in range(N_IMG):
                x_t = xpool.tile([P, NBLK, WP], f32, tag="x")
                nc.sync.dma_start(
                    out=x_t,
                    in_=xpad[st, 0:512, :].rearrange("(b q) w -> q b w", q=P),
                )
                bofs = st * 18
                d_tiles = []
                for t in range(9):
                    i, j = divmod(t, 3)
                    d = dpool.tile([P, NBLK, W], bf16, tag="d")
                    src = x_t[:, :, j : j + W]
                    if t in ACT_TAPS:
                        nc.scalar.activation(
                            out=d,
                            in_=src,
                            func=mybir.ActivationFunctionType.Abs,
                            bias=b_all[:, bofs + 9 + t : bofs + 10 + t],
                            scale=1.0,
                        )
                    else:
                        nc.vector._custom_dve(
                            absdiff,
                            out=d,
                            in0=src,
                            s0=b_all[:, bofs + t : bofs + t + 1],
                        )
                    d_tiles.append(d)

                # evacuate the PREVIOUS supertile's psum now — after this
                # supertile's taps were emitted — so ACT/DVE produce the next
                # tap batch before turning to evacuation and PE never starves
                # at the supertile boundary.
                if pending is not None:
                    _evac(*pending)
                    pending = None

                ps = ppool.tile([P, NBLK, W], f32, tag="ps")
                for i in range(3):
                    for j in range(3):
                        t = 3 * i + j
                        for blk in range(NBLK):
                            nc.tensor.matmul(
                                ps[:, blk, :],
                                lhsT=s_t[:, i, :],
                                rhs=d_tiles[t][:, blk, :],
                                start=(t == 0),
                                stop=(t == 8),
                            )
                pending = (ps, st)

            if pending is not None:
                _evac(*pending)
                pending = None
            for fi in range(2):
                _fixup(fi)

            # ---- fixup tiles: rows 127,128,255,256,383,384,511,512 (padded
            # coords) of each plane, 8 planes per tile. Partition layout:
            # q = 32*band + 4*g + r ; band b covers padded rows 126+128b .. 129+128b
    nc.finalize()
    return nc


def _get_program():
    if "nc" not in _PROGRAM_CACHE:
        _PROGRAM_CACHE["nc"] = _build_program()
    return _PROGRAM_CACHE["nc"]


def _host_consts(weight):
    """Shift matrices + per-partition bias tables (shared by all cores)."""
    w9 = np.asarray(weight, np.float32).reshape(C, 9)  # [c, t]

    S = np.zeros((3, P, P), np.float32)
    for i in range(3):
        for p in range(P):
            k = p + i - 1
            if 0 <= k < P:
                S[i, k, p] = 1.0
    S = S.astype(ml_dtypes.bfloat16)

    # one preloaded bias table [P, (N_IMG+2)*18]:
    # cols st*18+t = +w (DVE taps), st*18+9+t = -w (ACT taps)
    bias = np.zeros((P, (N_IMG + 2) * 18), np.float32)
    for st in range(N_IMG):  # main tiles: channel st % C, all partitions equal
        c = st % C
        bias[:, st * 18 : st * 18 + 9] = w9[c][None, :]
        bias[:, st * 18 + 9 : st * 18 + 18] = -w9[c][None, :]
    for fi in range(2):  # fixup tiles: partition q = 32*band + 4*g + r
        o = (N_IMG + fi) * 18
        for band in range(4):
            for g in range(8):
                c = (fi * 8 + g) % C
                lo = 32 * band + 4 * g
                bias[lo : lo + 4, o : o + 9] = w9[c][None, :]
                bias[lo : lo + 4, o + 9 : o + 18] = -w9[c][None, :]
    return S, bias


def kernel(input, weight):
    from concourse.bass_utils import run_bass_kernel_spmd

    x = np.asarray(input, np.float32)
    S, bias = _host_consts(weight)

    xpad = np.pad(x, ((0, 0), (0, 0), (1, 1), (1, 1)))  # [B, C, HP, WP]
    in_maps = []
    for core in range(N_CORES):
        shard = np.ascontiguousarray(
            xpad[core * B_LOC : (core + 1) * B_LOC].reshape(N_IMG, HP, WP)
        )
        in_maps.append({"xpad": shard, "smat": S, "bias": bias})

    nc = _get_program()
    res = run_bass_kernel_spmd(nc, in_maps, core_ids=list(range(N_CORES)))

    out = np.empty((B, C, H, W), np.float32)
    for core in range(N_CORES):
        o = res.results[core]["outp"].reshape(B_LOC, C, HP, W)
        out[core * B_LOC : (core + 1) * B_LOC] = o[:, :, 1 : H + 1, :]
    return out



# revision 3
# speedup vs baseline: 1.4939x; 1.4939x over previous
"""AdderNet depthwise 3x3 L1-distance conv for Trainium2, 8-core data parallel.

out[b,c,h,w] = -sum_{i,j in 3x3} |x_pad[b,c,h+i,w+j] - W[c,0,i,j]|

Strategy (per core, 4 images = 16 (b,c) planes):
- Host zero-pads each [512,512] plane to [514,514] and converts to bf16
  (halves input HBM traffic; rel-err budget is 2e-2, bf16 x costs ~4e-3).
- Per plane: 4 row-blocks of 128 rows on the 128 SBUF partitions.
- The three j-taps of each kernel row i are computed in ONE VectorE pass
  by a hand-written DVE uop program (FIR3_ANT): a 2-stage swap-flop delay
  line materializes x[n-1], x[n-2], then three ABSOLUTE_DIFF stages + adds
  produce |x[n]-w_i2| + |x[n-1]-w_i1| + |x[n-2]-w_i0| per element. 3 DVE
  passes replace 9 single-tap passes; the first 2 columns of each row are
  cross-row garbage and discarded (padded plane has 2 spare columns).
- The i row-shifts + the 3-plane sum run on TensorE with NEGATED shifted
  identity matrices accumulating in PSUM (so PSUM holds the final negated
  output; evacuation is a plain ScalarE copy).
- ScalarE additionally computes classic single-tap planes for the last
  (512-CSPLIT) output columns (activation Abs, bias=-w), balancing the
  DVE/ACT load; PE sums those with 9 small matmuls per block.
- Row-block seams (2 rows per 128-block) are patched by 2 small fixup
  tiles (baseline 9-tap path, channels vary per partition so the FIR's
  scalar immediates don't apply there).

Engine model per supertile (16 per core): DVE 3x(58+4*(CSPLIT+2))/0.96,
ACT 9x(224+4*(512-CSPLIT))/1.2 + evac (172+2048)/1.2, PE 48 MMs.
CSPLIT=440 balances DVE ~5.6us vs ACT ~5.7us; PE ~4.5us.
"""

import numpy as np
import ml_dtypes

B, C, H, W = 32, 4, 512, 512
N_CORES = 8
B_LOC = B // N_CORES          # 4 images per core
N_IMG = B_LOC * C             # 16 (b,c) planes per core
HP, WP = H + 2, W + 2         # 514, 514
NBLK = 4                      # row blocks of 128 per plane
P = 128
CSPLIT = 440                  # out cols [0,CSPLIT) via DVE FIR, rest via ACT
SW = W - CSPLIT

_PROGRAM_CACHE = {}


def _register_absdiff():
    """Custom DVE op: out = |in0 - s0| (for the seam-fixup tiles, where the
    channel varies per partition so s0 must be a per-partition scalar)."""
    from concourse import dve_ops
    from concourse.dve_spec import Spec, Src0, C0, relu, lower
    from concourse.dve_uop import DveOpSpec

    for o in dve_ops.OPS:
        if o.name == "ABS_DIFF_ANT":
            return o

    def _ref(in0, in1, s0, s1, imm2):
        s = np.asarray(s0)
        if s.ndim and in0.ndim > s.ndim:
            s = s.reshape(s.shape[0], *([1] * (in0.ndim - 1)))
        return np.abs(in0.astype(np.float32) - s).astype(np.float32)

    spec = Spec(body=relu(Src0 - C0) + relu(C0 - Src0), reference=_ref)
    shas = {
        ver: DveOpSpec(name="ABS_DIFF_ANT", uops=lower(spec, ver=ver)).sha(ver)
        for ver in ("v3", "v4")
    }
    op = dve_ops.DveOp("ABS_DIFF_ANT", spec, subdim=False, uops_sha=shas)
    dve_ops.OPS.append(op)
    dve_ops.CUSTOM_DVE_SPECS[op.name] = spec
    dve_ops._SUB_OPCODE_FOR_NAME[op.name] = (
        dve_ops._CUSTOM_DVE_ROW_BASE + len(dve_ops.OPS) - 1
    )
    return op


def _fir_ref(in0, in1, s0, s1, imm2):
    x = np.asarray(in0, np.float32)
    x1 = np.roll(x, 1, axis=-1)
    x2 = np.roll(x, 2, axis=-1)
    x1[..., 0] = 0
    x2[..., :2] = 0
    return (
        np.abs(x - np.float32(s0))
        + np.abs(x1 - np.float32(s1))
        + np.abs(x2 - np.float32(imm2))
    ).astype(np.float32)


def _register_fir3():
    """Hand-written DVE uop program (bypasses lower()):
    out[n] = |x[n]-s0| + |x[n-1]-s1| + |x[n-2]-imm2|.
    Swap flops at blocks 0/1 act as a 2-deep delay line (BYPASS outputs A
    and latches B; CURR_SWAP_OUT reads last cycle's latch), then three
    ABSOLUTE_DIFF ALU stages + two ADDs. Validated on HW (exp_fir.py)."""
    from concourse import dve_ops
    from concourse.dve_spec import Spec, Src0, C0, C1, C2
    from concourse.dve_uop import (
        DveOpSpec,
        UopConfig,
        AluOp,
        AluInp,
        InpSel,
        DelayInp,
        OutSel,
        OutPath,
        Trigger,
        ENABLE,
    )

    for o in dve_ops.OPS:
        if o.name == "FIR3_ANT":
            return o

    u = UopConfig()
    u.enable_input(InpSel.SRC_0, 1)      # chain0 @B0 = x[n]
    u.enable_input(InpSel.CONST_0, 2)    # chain1 = c0
    u.enable_input(InpSel.CONST_1, 3)    # chain2 = c1
    u.enable_input(InpSel.CONST_2, 4)    # chain3 = c2
    u.require_inp0 = ENABLE
    u.trigger = (Trigger.SRC_TENSOR_DONE, Trigger.NONE, Trigger.NONE)
    u.next_uop = (0, 0, 0)
    u.enable_output(OutSel.ALU_OUT, OutPath.WR0_LO)
    dp = u.datapath_config
    dp[0].enable_alu(AluOp.BYPASS, AluInp.CURR_SWAP_OUT, AluInp.PREV_DELAY_0)
    dp[0].swap_enable = ENABLE           # out = x[n-1], latch x[n]
    dp[0].pass_through_delay(0, 1, 2, 3)
    dp[1].enable_alu(AluOp.BYPASS, AluInp.CURR_SWAP_OUT, AluInp.PREV_ALU_OUT)
    dp[1].swap_enable = ENABLE           # out = x[n-2], latch x[n-1]
    dp[1].pass_through_delay(0, 1, 2, 3)
    dp[1].enable_delay_from_src(DelayInp.PREV_ALU_OUT, 4)  # lane4 = x[n-1]
    dp[2].enable_alu(AluOp.ABSOLUTE_DIFF, AluInp.PREV_ALU_OUT, AluInp.PREV_DELAY_3)
    dp[2].pass_through_delay(0, 1, 2, 4)  # t2 = |x[n-2]-c2|
    dp[3].enable_alu(AluOp.ABSOLUTE_DIFF, AluInp.PREV_DELAY_4, AluInp.PREV_DELAY_2)
    dp[3].pass_through_delay(0, 1)        # t1 = |x[n-1]-c1|
    dp[3].enable_delay_from_src(DelayInp.PREV_ALU_OUT, 5)  # lane5 = t2
    dp[4].enable_alu(AluOp.ADD, AluInp.PREV_ALU_OUT, AluInp.PREV_DELAY_5)
    dp[4].pass_through_delay(0, 1)        # t12 = t1 + t2
    dp[5].enable_alu(AluOp.ABSOLUTE_DIFF, AluInp.PREV_DELAY_0, AluInp.PREV_DELAY_1)
    dp[5].enable_delay_from_src(DelayInp.PREV_ALU_OUT, 2)  # t0; lane2 = t12
    dp[6].enable_alu(AluOp.ADD, AluInp.PREV_ALU_OUT, AluInp.PREV_DELAY_2)
    dp[7].pass_through_alu()
    u.validate("v3")

    row = dve_ops._CUSTOM_DVE_ROW_BASE + len(dve_ops.OPS)
    raw = DveOpSpec(name="FIR3_ANT", uops=[u], opcode=row, rd1_en=False)
    raw.validate("v3")
    spec = Spec(body=(Src0 - C0) + C1 + C2, reference=_fir_ref)
    op = dve_ops.DveOp(
        "FIR3_ANT",
        spec,
        subdim=False,
        uops_sha={v: raw.sha(v) for v in ("v3", "v4")},
    )
    dve_ops.OPS.append(op)
    dve_ops.CUSTOM_DVE_SPECS[op.name] = spec
    dve_ops._SUB_OPCODE_FOR_NAME[op.name] = row
    for ver in ("v3", "v4"):
        dve_ops._COMPILE_CACHE[("FIR3_ANT", ver)] = raw
    return op


# tap t = 3*i + j ; fixup taps computed on ScalarE (rest on VectorE absdiff)
ACT_TAPS = (0, 2, 4, 6, 8)


def _patch_ldw_opt():
    """concourse hardcodes walrus --enable-ldw-opt=false; without it every
    matmul pays a serialized LDWEIGHTS reload (~384ns/MM measured vs ~216
    streaming). Our loop issues 16 consecutive matmuls per stationary shift
    matrix, so the dedup pass eliminates 45 of 48 LDWs per supertile."""
    import concourse.bass_utils as bu

    if getattr(bu, "_ldw_patched", False):
        return
    orig = bu.run_command

    def patched(argv, **kw):
        argv = [
            "--enable-ldw-opt=true" if a == "--enable-ldw-opt=false" else a
            for a in argv
        ]
        return orig(argv, **kw)

    bu.run_command = patched
    bu._ldw_patched = True


def _build_program(w9):
    """w9: [C, 9] float32 weight taps (baked into FIR immediates)."""
    import concourse.mybir as mybir
    import concourse.tile as tile
    from concourse import bacc

    f32 = mybir.dt.float32
    bf16 = mybir.dt.bfloat16
    absdiff = _register_absdiff()
    fir3 = _register_fir3()
    nc = bacc.Bacc("TRN2", target_bir_lowering=False)

    xpad = nc.declare_dram_parameter("xpad", [N_IMG, HP, WP], bf16, isOutput=False)
    smat = nc.declare_dram_parameter("smat", [3, P, P], bf16, isOutput=False)
    bias = nc.declare_dram_parameter("bias", [P, (N_IMG + 2) * 18], f32, isOutput=False)
    outp = nc.declare_dram_parameter("outp", [N_IMG, HP, W], f32, isOutput=True)

    with tile.TileContext(nc) as tc:
        with (
            tc.tile_pool(name="const", bufs=1) as cpool,
            tc.tile_pool(name="xp", bufs=3) as xpool,
            tc.tile_pool(name="fp", bufs=6) as fpool,
            tc.tile_pool(name="dp", bufs=18) as dpool,
            tc.tile_pool(name="op", bufs=4) as opool,
            tc.tile_pool(name="ps", bufs=2, space="PSUM") as ppool,
        ):
            s_t = cpool.tile([P, 3, P], bf16, tag="s")
            nc.sync.dma_start(out=s_t, in_=smat[:].rearrange("s k p -> k s p"))
            b_all = cpool.tile([P, (N_IMG + 2) * 18], f32, tag="ball")
            nc.sync.dma_start(out=b_all, in_=bias[:])

            # Warmup: attach the ACT table-load pseudo-instruction here.
            warm = cpool.tile([P, 2], f32, tag="warm")
            nc.vector.memset(warm, 0.0)
            nc.scalar.activation(
                out=warm[:, 0:1],
                in_=warm[:, 1:2],
                func=mybir.ActivationFunctionType.Abs,
                bias=0.0,
                scale=1.0,
            )
            nc.scalar.activation(
                out=warm[:, 1:2],
                in_=warm[:, 0:1],
                func=mybir.ActivationFunctionType.Copy,
                scale=-1.0,
            )

            def _evac(ps, st):
                o_t = opool.tile([P, NBLK, W], f32, tag="o")
                nc.scalar.activation(
                    out=o_t,
                    in_=ps,
                    func=mybir.ActivationFunctionType.Copy,
                    scale=1.0,
                )
                nc.sync.dma_start(
                    out=outp[st, 0:512, :].rearrange("(b q) w -> q b w", q=P)[1:127],
                    in_=o_t[1:127],
                )

            def _fixup(fi):
                g0 = fi * 8
                xf = xpool.tile([P, WP], bf16, tag="xf")
                for band in range(4):
                    nc.sync.dma_start(
                        out=xf[32 * band : 32 * (band + 1)],
                        in_=xpad[g0 : g0 + 8, 126 + 128 * band : 130 + 128 * band, :],
                    )
                bofs = (N_IMG + fi) * 18
                df_tiles = []
                for t in range(9):
                    i, j = divmod(t, 3)
                    d = dpool.tile([P, W], bf16, tag="df")
                    src = xf[:, j : j + W]
                    if t in ACT_TAPS:
                        nc.scalar.activation(
                            out=d,
                            in_=src,
                            func=mybir.ActivationFunctionType.Abs,
                            bias=b_all[:, bofs + 9 + t : bofs + 10 + t],
                            scale=1.0,
                        )
                    else:
                        nc.vector._custom_dve(
                            absdiff,
                            out=d,
                            in0=src,
                            s0=b_all[:, bofs + t : bofs + t + 1],
                        )
                    df_tiles.append(d)

                pf = ppool.tile([P, W], mybir.dt.float32, tag="ps")
                for i in range(3):
                    t0 = 3 * i
                    for j in range(3):
                        nc.tensor.matmul(
                            pf,
                            lhsT=s_t[:, i, :],
                            rhs=df_tiles[t0 + j],
                            start=(t0 + j == 0),
                            stop=(t0 + j == 8),
                        )

                of = opool.tile([P, W], f32, tag="o")
                nc.scalar.activation(
                    out=of,
                    in_=pf,
                    func=mybir.ActivationFunctionType.Copy,
                    scale=1.0,
                )
                for band in range(4):
                    for g in range(8):
                        lo = 32 * band + 4 * g
                        nc.sync.dma_start(
                            out=outp[g0 + g, 127 + 128 * band : 129 + 128 * band, :],
                            in_=of[lo + 1 : lo + 3],
                        )

            pending = None
            # ---- main supertiles: one per (b,c) plane ----
            for st in range(N_IMG):
                c = st % C
                x_t = xpool.tile([P, NBLK, WP], bf16, tag="x")
                nc.sync.dma_start(
                    out=x_t,
                    in_=xpad[st, 0:512, :].rearrange("(b q) w -> q b w", q=P),
                )
                # DVE: 3 FIR passes (all 3 j-taps of kernel row i fused)
                p_tiles = []
                for i in range(3):
                    pt = fpool.tile([P, NBLK, CSPLIT + 2], bf16, tag="f")
                    nc.vector._custom_dve(
                        fir3,
                        out=pt,
                        in0=x_t[:, :, 0 : CSPLIT + 2],
                        s0=float(w9[c, 3 * i + 2]),
                        s1=float(w9[c, 3 * i + 1]),
                        imm2=float(w9[c, 3 * i + 0]),
                    )
                    p_tiles.append(pt)
                # ACT: single-tap planes for the last SW output columns
                bofs = st * 18
                d_tiles = []
                for t in range(9):
                    i, j = divmod(t, 3)
                    d = dpool.tile([P, NBLK, SW], bf16, tag="d")
                    nc.scalar.activation(
                        out=d,
                        in_=x_t[:, :, CSPLIT + j : CSPLIT + j + SW],
                        func=mybir.ActivationFunctionType.Abs,
                        bias=b_all[:, bofs + 9 + t : bofs + 10 + t],
                        scale=1.0,
                    )
                    d_tiles.append(d)

                # evacuate the PREVIOUS supertile's psum now, after this
                # supertile's engine work was queued, so ACT stays busy and
                # PE never starves at the supertile boundary.
                if pending is not None:
                    _evac(*pending)
                    pending = None

                ps = ppool.tile([P, NBLK, W], mybir.dt.float32, tag="ps")
                for i in range(3):
                    for blk in range(NBLK):
                        nc.tensor.matmul(
                            ps[:, blk, 0:CSPLIT],
                            lhsT=s_t[:, i, :],
                            rhs=p_tiles[i][:, blk, 2 : CSPLIT + 2],
                            start=(i == 0),
                            stop=False,
                        )
                        for j in range(3):
                            nc.tensor.matmul(
                                ps[:, blk, CSPLIT:W],
                                lhsT=s_t[:, i, :],
                                rhs=d_tiles[3 * i + j][:, blk, :],
                                start=False,
                                stop=(i == 2 and j == 2),
                            )
                pending = (ps, st)

            if pending is not None:
                _evac(*pending)
                pending = None
            for fi in range(2):
                _fixup(fi)
    nc.finalize()
    return nc


def _get_program(w9):
    key = w9.tobytes()
    if _PROGRAM_CACHE.get("key") != key:
        _PROGRAM_CACHE["nc"] = _build_program(w9)
        _PROGRAM_CACHE["key"] = key
    return _PROGRAM_CACHE["nc"]


def _host_consts(weight):
    """Negated shift matrices + per-partition bias tables (shared by cores)."""
    w9 = np.asarray(weight, np.float32).reshape(C, 9)  # [c, t]

    S = np.zeros((3, P, P), np.float32)
    for i in range(3):
        for p in range(P):
            k = p + i - 1
            if 0 <= k < P:
                S[i, k, p] = -1.0
    S = S.astype(ml_dtypes.bfloat16)

    # bias table [P, (N_IMG+2)*18]: cols st*18+t = +w (DVE absdiff taps),
    # st*18+9+t = -w (ACT taps); supertiles 0-15 then fixup tiles 16-17
    bias = np.zeros((P, (N_IMG + 2) * 18), np.float32)
    for st in range(N_IMG):
        c = st % C
        bias[:, st * 18 : st * 18 + 9] = w9[c][None, :]
        bias[:, st * 18 + 9 : st * 18 + 18] = -w9[c][None, :]
    for fi in range(2):  # fixup tiles: partition q = 32*band + 4*g + r
        o = (N_IMG + fi) * 18
        for band in range(4):
            for g in range(8):
                c = (fi * 8 + g) % C
                lo = 32 * band + 4 * g
                bias[lo : lo + 4, o : o + 9] = w9[c][None, :]
                bias[lo : lo + 4, o + 9 : o + 18] = -w9[c][None, :]
    return S, bias, w9


def kernel(input, weight):
    from concourse.bass_utils import run_bass_kernel_spmd

    x = np.asarray(input, np.float32)
    S, bias, w9 = _host_consts(weight)

    xpad = np.pad(x, ((0, 0), (0, 0), (1, 1), (1, 1))).astype(ml_dtypes.bfloat16)
    in_maps = []
    for core in range(N_CORES):
        shard = np.ascontiguousarray(
            xpad[core * B_LOC : (core + 1) * B_LOC].reshape(N_IMG, HP, WP)
        )
        in_maps.append({"xpad": shard, "smat": S, "bias": bias})

    nc = _get_program(w9)
    res = run_bass_kernel_spmd(nc, in_maps, core_ids=list(range(N_CORES)))

    out = np.empty((B, C, H, W), np.float32)
    for core in range(N_CORES):
        o = res.results[core]["outp"].reshape(B_LOC, C, HP, W)
        out[core * B_LOC : (core + 1) * B_LOC] = o[:, :, 1 : H + 1, :]
    return out


# revision 8
# speedup vs baseline: 1.4967x; 1.0018x over previous
"""AdderNet depthwise 3x3 L1-distance conv for Trainium2, 8-core data parallel.

out[b,c,h,w] = -sum_{i,j in 3x3} |x_pad[b,c,h+i,w+j] - W[c,0,i,j]|

Strategy (per core, 4 images = 16 (b,c) planes):
- Host zero-pads each [512,512] plane to [514,514] and converts to bf16
  (halves input HBM traffic; rel-err budget is 2e-2, bf16 x costs ~4e-3).
- Per plane: 4 row-blocks of 128 rows on the 128 SBUF partitions.
- The three j-taps of each kernel row i are computed in ONE VectorE pass
  by a hand-written DVE uop program (FIR3_ANT): a 2-stage swap-flop delay
  line materializes x[n-1], x[n-2], then three ABSOLUTE_DIFF stages + adds
  produce |x[n]-w_i2| + |x[n-1]-w_i1| + |x[n-2]-w_i0| per element. 3 DVE
  passes replace 9 single-tap passes; the first 2 columns of each row are
  cross-row garbage and discarded (padded plane has 2 spare columns).
- The i row-shifts + the 3-plane sum run on TensorE with NEGATED shifted
  identity matrices accumulating in PSUM (so PSUM holds the final negated
  output; evacuation is a plain ScalarE copy).
- ScalarE additionally computes classic single-tap planes for the last
  (512-CSPLIT) output columns (activation Abs, bias=-w), balancing the
  DVE/ACT load; PE sums those with 9 small matmuls per block.
- Row-block seams (2 rows per 128-block) are patched by 2 small fixup
  tiles (baseline 9-tap path, channels vary per partition so the FIR's
  scalar immediates don't apply there).

Engine model per supertile (16 per core): DVE 3x(58+4*(CSPLIT+2))/0.96,
ACT 9x(224+4*(512-CSPLIT))/1.2 + evac (172+2048)/1.2, PE 48 MMs.
CSPLIT=440 balances DVE ~5.6us vs ACT ~5.7us; PE ~4.5us.
"""

import numpy as np
import ml_dtypes

B, C, H, W = 32, 4, 512, 512
N_CORES = 8
B_LOC = B // N_CORES          # 4 images per core
N_IMG = B_LOC * C             # 16 (b,c) planes per core
HP, WP = H + 2, W + 2         # 514, 514
NBLK = 4                      # row blocks of 128 per plane
P = 128
CSPLIT = 440                  # out cols [0,CSPLIT) via DVE FIR, rest via ACT
SW = W - CSPLIT

_PROGRAM_CACHE = {}


def _register_absdiff():
    """Custom DVE op: out = |in0 - s0| (for the seam-fixup tiles, where the
    channel varies per partition so s0 must be a per-partition scalar)."""
    from concourse import dve_ops
    from concourse.dve_spec import Spec, Src0, C0, relu, lower
    from concourse.dve_uop import DveOpSpec

    for o in dve_ops.OPS:
        if o.name == "ABS_DIFF_ANT":
            return o

    def _ref(in0, in1, s0, s1, imm2):
        s = np.asarray(s0)
        if s.ndim and in0.ndim > s.ndim:
            s = s.reshape(s.shape[0], *([1] * (in0.ndim - 1)))
        return np.abs(in0.astype(np.float32) - s).astype(np.float32)

    spec = Spec(body=relu(Src0 - C0) + relu(C0 - Src0), reference=_ref)
    shas = {
        ver: DveOpSpec(name="ABS_DIFF_ANT", uops=lower(spec, ver=ver)).sha(ver)
        for ver in ("v3", "v4")
    }
    op = dve_ops.DveOp("ABS_DIFF_ANT", spec, subdim=False, uops_sha=shas)
    dve_ops.OPS.append(op)
    dve_ops.CUSTOM_DVE_SPECS[op.name] = spec
    dve_ops._SUB_OPCODE_FOR_NAME[op.name] = (
        dve_ops._CUSTOM_DVE_ROW_BASE + len(dve_ops.OPS) - 1
    )
    return op


def _fir_ref(in0, in1, s0, s1, imm2):
    x = np.asarray(in0, np.float32)
    x1 = np.roll(x, 1, axis=-1)
    x2 = np.roll(x, 2, axis=-1)
    x1[..., 0] = 0
    x2[..., :2] = 0
    return (
        np.abs(x - np.float32(s0))
        + np.abs(x1 - np.float32(s1))
        + np.abs(x2 - np.float32(imm2))
    ).astype(np.float32)


def _register_fir3():
    """Hand-written DVE uop program (bypasses lower()):
    out[n] = |x[n]-s0| + |x[n-1]-s1| + |x[n-2]-imm2|.
    Swap flops at blocks 0/1 act as a 2-deep delay line (BYPASS outputs A
    and latches B; CURR_SWAP_OUT reads last cycle's latch), then three
    ABSOLUTE_DIFF ALU stages + two ADDs. Validated on HW (exp_fir.py)."""
    from concourse import dve_ops
    from concourse.dve_spec import Spec, Src0, C0, C1, C2
    from concourse.dve_uop import (
        DveOpSpec,
        UopConfig,
        AluOp,
        AluInp,
        InpSel,
        DelayInp,
        OutSel,
        OutPath,
        Trigger,
        ENABLE,
    )

    for o in dve_ops.OPS:
        if o.name == "FIR3_ANT":
            return o

    u = UopConfig()
    u.enable_input(InpSel.SRC_0, 1)      # chain0 @B0 = x[n]
    u.enable_input(InpSel.CONST_0, 2)    # chain1 = c0
    u.enable_input(InpSel.CONST_1, 3)    # chain2 = c1
    u.enable_input(InpSel.CONST_2, 4)    # chain3 = c2
    u.require_inp0 = ENABLE
    u.trigger = (Trigger.SRC_TENSOR_DONE, Trigger.NONE, Trigger.NONE)
    u.next_uop = (0, 0, 0)
    u.enable_output(OutSel.ALU_OUT, OutPath.WR0_LO)
    dp = u.datapath_config
    dp[0].enable_alu(AluOp.BYPASS, AluInp.CURR_SWAP_OUT, AluInp.PREV_DELAY_0)
    dp[0].swap_enable = ENABLE           # out = x[n-1], latch x[n]
    dp[0].pass_through_delay(0, 1, 2, 3)
    dp[1].enable_alu(AluOp.BYPASS, AluInp.CURR_SWAP_OUT, AluInp.PREV_ALU_OUT)
    dp[1].swap_enable = ENABLE           # out = x[n-2], latch x[n-1]
    dp[1].pass_through_delay(0, 1, 2, 3)
    dp[1].enable_delay_from_src(DelayInp.PREV_ALU_OUT, 4)  # lane4 = x[n-1]
    dp[2].enable_alu(AluOp.ABSOLUTE_DIFF, AluInp.PREV_ALU_OUT, AluInp.PREV_DELAY_3)
    dp[2].pass_through_delay(0, 1, 2, 4)  # t2 = |x[n-2]-c2|
    dp[3].enable_alu(AluOp.ABSOLUTE_DIFF, AluInp.PREV_DELAY_4, AluInp.PREV_DELAY_2)
    dp[3].pass_through_delay(0, 1)        # t1 = |x[n-1]-c1|
    dp[3].enable_delay_from_src(DelayInp.PREV_ALU_OUT, 5)  # lane5 = t2
    dp[4].enable_alu(AluOp.ADD, AluInp.PREV_ALU_OUT, AluInp.PREV_DELAY_5)
    dp[4].pass_through_delay(0, 1)        # t12 = t1 + t2
    dp[5].enable_alu(AluOp.ABSOLUTE_DIFF, AluInp.PREV_DELAY_0, AluInp.PREV_DELAY_1)
    dp[5].enable_delay_from_src(DelayInp.PREV_ALU_OUT, 2)  # t0; lane2 = t12
    dp[6].enable_alu(AluOp.ADD, AluInp.PREV_ALU_OUT, AluInp.PREV_DELAY_2)
    dp[7].pass_through_alu()
    u.validate("v3")

    row = dve_ops._CUSTOM_DVE_ROW_BASE + len(dve_ops.OPS)
    raw = DveOpSpec(name="FIR3_ANT", uops=[u], opcode=row, rd1_en=False)
    raw.validate("v3")
    spec = Spec(body=(Src0 - C0) + C1 + C2, reference=_fir_ref)
    op = dve_ops.DveOp(
        "FIR3_ANT",
        spec,
        subdim=False,
        uops_sha={v: raw.sha(v) for v in ("v3", "v4")},
    )
    dve_ops.OPS.append(op)
    dve_ops.CUSTOM_DVE_SPECS[op.name] = spec
    dve_ops._SUB_OPCODE_FOR_NAME[op.name] = row
    for ver in ("v3", "v4"):
        dve_ops._COMPILE_CACHE[("FIR3_ANT", ver)] = raw
    return op


# tap t = 3*i + j ; fixup taps computed on ScalarE (rest on VectorE absdiff)
ACT_TAPS = (0, 2, 4, 6, 8)


def _matmul_noldw(nc, **kw):
    """Emit a matmul that SKIPS its implicit LDWEIGHTS (InstMatmult.ldweights
    =False), relying on an explicit nc.tensor.ldweights() issued earlier.
    Without this every matmul pays a serialized ~98ns LDW reload plus issue
    gaps (~384ns/MM measured for N=512 vs ~216 streaming). Only valid for
    16-bit weights (f32/f32r standalone-LDW is broken in walrus codegen).
    bacc's move_matmul_waits_to_ldweights pass handles the sem-wait hoist."""
    import concourse.mybir as mybir

    orig = mybir.InstMatmult

    def patched(**kw2):
        kw2["ldweights"] = False
        return orig(**kw2)

    mybir.InstMatmult = patched
    try:
        return nc.tensor.matmul(**kw)
    finally:
        mybir.InstMatmult = orig


def _build_program(w9):
    """w9: [C, 9] float32 weight taps (baked into FIR immediates)."""
    import concourse.mybir as mybir
    import concourse.tile as tile
    from concourse import bacc

    f32 = mybir.dt.float32
    bf16 = mybir.dt.bfloat16
    absdiff = _register_absdiff()
    fir3 = _register_fir3()
    nc = bacc.Bacc("TRN2", target_bir_lowering=False)

    xpad = nc.declare_dram_parameter("xpad", [N_IMG, HP, WP], bf16, isOutput=False)
    smat = nc.declare_dram_parameter("smat", [3, P, P], bf16, isOutput=False)
    bias = nc.declare_dram_parameter("bias", [P, (N_IMG + 2) * 18], f32, isOutput=False)
    outp = nc.declare_dram_parameter("outp", [N_IMG, HP, W], f32, isOutput=True)

    with tile.TileContext(nc) as tc:
        with (
            tc.tile_pool(name="const", bufs=1) as cpool,
            tc.tile_pool(name="xp", bufs=3) as xpool,
            tc.tile_pool(name="fp", bufs=6) as fpool,
            tc.tile_pool(name="dp", bufs=18) as dpool,
            tc.tile_pool(name="op", bufs=4) as opool,
            tc.tile_pool(name="ps", bufs=2, space="PSUM") as ppool,
        ):
            s_t = cpool.tile([P, 3, P], bf16, tag="s")
            nc.sync.dma_start(out=s_t, in_=smat[:].rearrange("s k p -> k s p"))
            b_all = cpool.tile([P, (N_IMG + 2) * 18], f32, tag="ball")
            nc.sync.dma_start(out=b_all, in_=bias[:])

            # Warmup: attach the ACT table-load pseudo-instruction here.
            warm = cpool.tile([P, 2], f32, tag="warm")
            nc.vector.memset(warm, 0.0)
            nc.scalar.activation(
                out=warm[:, 0:1],
                in_=warm[:, 1:2],
                func=mybir.ActivationFunctionType.Abs,
                bias=0.0,
                scale=1.0,
            )
            nc.scalar.activation(
                out=warm[:, 1:2],
                in_=warm[:, 0:1],
                func=mybir.ActivationFunctionType.Copy,
                scale=-1.0,
            )

            def _evac(ps, st):
                o_t = opool.tile([P, NBLK, W], f32, tag="o")
                nc.scalar.activation(
                    out=o_t,
                    in_=ps,
                    func=mybir.ActivationFunctionType.Copy,
                    scale=1.0,
                )
                nc.sync.dma_start(
                    out=outp[st, 0:512, :].rearrange("(b q) w -> q b w", q=P)[1:127],
                    in_=o_t[1:127],
                )

            def _fixup(fi):
                g0 = fi * 8
                xf = xpool.tile([P, WP], bf16, tag="xf")
                for band in range(4):
                    nc.sync.dma_start(
                        out=xf[32 * band : 32 * (band + 1)],
                        in_=xpad[g0 : g0 + 8, 126 + 128 * band : 130 + 128 * band, :],
                    )
                bofs = (N_IMG + fi) * 18
                df_tiles = []
                for t in range(9):
                    i, j = divmod(t, 3)
                    d = dpool.tile([P, W], bf16, tag="df")
                    src = xf[:, j : j + W]
                    if t in ACT_TAPS:
                        nc.scalar.activation(
                            out=d,
                            in_=src,
                            func=mybir.ActivationFunctionType.Abs,
                            bias=b_all[:, bofs + 9 + t : bofs + 10 + t],
                            scale=1.0,
                        )
                    else:
                        nc.vector._custom_dve(
                            absdiff,
                            out=d,
                            in0=src,
                            s0=b_all[:, bofs + t : bofs + t + 1],
                        )
                    df_tiles.append(d)

                pf = ppool.tile([P, W], mybir.dt.float32, tag="ps")
                for i in range(3):
                    t0 = 3 * i
                    nc.tensor.ldweights(s_t[:, i, :])
                    for j in range(3):
                        _matmul_noldw(
                            nc,
                            out=pf,
                            lhsT=s_t[:, i, :],
                            rhs=df_tiles[t0 + j],
                            start=(t0 + j == 0),
                            stop=(t0 + j == 8),
                        )

                of = opool.tile([P, W], f32, tag="o")
                nc.scalar.activation(
                    out=of,
                    in_=pf,
                    func=mybir.ActivationFunctionType.Copy,
                    scale=1.0,
                )
                for band in range(4):
                    for g in range(8):
                        lo = 32 * band + 4 * g
                        nc.sync.dma_start(
                            out=outp[g0 + g, 127 + 128 * band : 129 + 128 * band, :],
                            in_=of[lo + 1 : lo + 3],
                        )

            pending = None
            # ---- main supertiles: one per (b,c) plane ----
            for st in range(N_IMG):
                c = st % C
                x_t = xpool.tile([P, NBLK, WP], bf16, tag="x")
                nc.sync.dma_start(
                    out=x_t,
                    in_=xpad[st, 0:512, :].rearrange("(b q) w -> q b w", q=P),
                )
                # DVE: 3 FIR passes (all 3 j-taps of kernel row i fused)
                p_tiles = []
                for i in range(3):
                    pt = fpool.tile([P, NBLK, CSPLIT + 2], bf16, tag="f")
                    nc.vector._custom_dve(
                        fir3,
                        out=pt,
                        in0=x_t[:, :, 0 : CSPLIT + 2],
                        s0=float(w9[c, 3 * i + 2]),
                        s1=float(w9[c, 3 * i + 1]),
                        imm2=float(w9[c, 3 * i + 0]),
                    )
                    p_tiles.append(pt)
                # ACT: single-tap planes for the last SW output columns
                bofs = st * 18
                d_tiles = []
                for t in range(9):
                    i, j = divmod(t, 3)
                    d = dpool.tile([P, NBLK, SW], bf16, tag="d")
                    nc.scalar.activation(
                        out=d,
                        in_=x_t[:, :, CSPLIT + j : CSPLIT + j + SW],
                        func=mybir.ActivationFunctionType.Abs,
                        bias=b_all[:, bofs + 9 + t : bofs + 10 + t],
                        scale=1.0,
                    )
                    d_tiles.append(d)

                # evacuate the PREVIOUS supertile's psum now, after this
                # supertile's engine work was queued, so ACT stays busy and
                # PE never starves at the supertile boundary.
                if pending is not None:
                    _evac(*pending)
                    pending = None

                ps = ppool.tile([P, NBLK, W], mybir.dt.float32, tag="ps")
                for i in range(3):
                    nc.tensor.ldweights(s_t[:, i, :])
                    for blk in range(NBLK):
                        _matmul_noldw(
                            nc,
                            out=ps[:, blk, 0:CSPLIT],
                            lhsT=s_t[:, i, :],
                            rhs=p_tiles[i][:, blk, 2 : CSPLIT + 2],
                            start=(i == 0),
                            stop=False,
                        )
                        for j in range(3):
                            _matmul_noldw(
                                nc,
                                out=ps[:, blk, CSPLIT:W],
                                lhsT=s_t[:, i, :],
                                rhs=d_tiles[3 * i + j][:, blk, :],
                                start=False,
                                stop=(i == 2 and j == 2),
                            )
                pending = (ps, st)

            if pending is not None:
                _evac(*pending)
                pending = None
            for fi in range(2):
                _fixup(fi)
    nc.finalize()
    return nc


def _get_program(w9):
    key = w9.tobytes()
    if _PROGRAM_CACHE.get("key") != key:
        _PROGRAM_CACHE["nc"] = _build_program(w9)
        _PROGRAM_CACHE["key"] = key
    return _PROGRAM_CACHE["nc"]


def _host_consts(weight):
    """Negated shift matrices + per-partition bias tables (shared by cores)."""
    w9 = np.asarray(weight, np.float32).reshape(C, 9)  # [c, t]

    S = np.zeros((3, P, P), np.float32)
    for i in range(3):
        for p in range(P):
            k = p + i - 1
            if 0 <= k < P:
                S[i, k, p] = -1.0
    S = S.astype(ml_dtypes.bfloat16)

    # bias table [P, (N_IMG+2)*18]: cols st*18+t = +w (DVE absdiff taps),
    # st*18+9+t = -w (ACT taps); supertiles 0-15 then fixup tiles 16-17
    bias = np.zeros((P, (N_IMG + 2) * 18), np.float32)
    for st in range(N_IMG):
        c = st % C
        bias[:, st * 18 : st * 18 + 9] = w9[c][None, :]
        bias[:, st * 18 + 9 : st * 18 + 18] = -w9[c][None, :]
    for fi in range(2):  # fixup tiles: partition q = 32*band + 4*g + r
        o = (N_IMG + fi) * 18
        for band in range(4):
            for g in range(8):
                c = (fi * 8 + g) % C
                lo = 32 * band + 4 * g
                bias[lo : lo + 4, o : o + 9] = w9[c][None, :]
                bias[lo : lo + 4, o + 9 : o + 18] = -w9[c][None, :]
    return S, bias, w9


def kernel(input, weight):
    from concourse.bass_utils import run_bass_kernel_spmd

    x = np.asarray(input, np.float32)
    S, bias, w9 = _host_consts(weight)

    xpad = np.pad(x, ((0, 0), (0, 0), (1, 1), (1, 1))).astype(ml_dtypes.bfloat16)
    in_maps = []
    for core in range(N_CORES):
        shard = np.ascontiguousarray(
            xpad[core * B_LOC : (core + 1) * B_LOC].reshape(N_IMG, HP, WP)
        )
        in_maps.append({"xpad": shard, "smat": S, "bias": bias})

    nc = _get_program(w9)
    res = run_bass_kernel_spmd(nc, in_maps, core_ids=list(range(N_CORES)))

    out = np.empty((B, C, H, W), np.float32)
    for core in range(N_CORES):
        o = res.results[core]["outp"].reshape(B_LOC, C, HP, W)
        out[core * B_LOC : (core + 1) * B_LOC] = o[:, :, 1 : H + 1, :]
    return out


# revision 15
# speedup vs baseline: 1.9142x; 1.2790x over previous
"""AdderNet depthwise 3x3 L1-distance conv for Trainium2, 8-core data parallel.

out[b,c,h,w] = -sum_{i,j in 3x3} |x_pad[b,c,h+i,w+j] - W[c,0,i,j]|

Strategy (per core, 4 images = 16 (b,c) planes):
- Host zero-pads each [512,512] plane to [514,514] and converts to bf16
  (halves input HBM traffic; rel-err budget is 2e-2, bf16 x costs ~4e-3).
- Per plane: 4 row-blocks of 128 rows on the 128 SBUF partitions.
- The three j-taps of each kernel row i are computed in ONE VectorE pass
  by a hand-written DVE uop program (FIR3_ANT): a 2-stage swap-flop delay
  line materializes x[n-1], x[n-2], then three ABSOLUTE_DIFF stages + adds
  produce |x[n]-w_i2| + |x[n-1]-w_i1| + |x[n-2]-w_i0| per element. 3 DVE
  passes replace 9 single-tap passes; the first 2 columns of each row are
  cross-row garbage and discarded (padded plane has 2 spare columns).
- The i row-shifts + the 3-plane sum run on TensorE with NEGATED shifted
  identity matrices accumulating in PSUM (so PSUM holds the final negated
  output; evacuation is a plain ScalarE copy).
- ScalarE additionally computes classic single-tap planes for the last
  (512-CSPLIT) output columns (activation Abs, bias=-w), balancing the
  DVE/ACT load; PE sums those with 9 small matmuls per block.
- Row-block seams (2 rows per 128-block) are patched by 2 small fixup
  tiles (baseline 9-tap path, channels vary per partition so the FIR's
  scalar immediates don't apply there).

Engine model per supertile (16 per core): DVE 3x(58+4*(CSPLIT+2))/0.96,
ACT 9x(224+4*(512-CSPLIT))/1.2 + evac (172+2048)/1.2, PE 48 MMs.
CSPLIT=440 balances DVE ~5.6us vs ACT ~5.7us; PE ~4.5us.
"""

import numpy as np
import ml_dtypes

B, C, H, W = 32, 4, 512, 512
N_CORES = 8
B_LOC = B // N_CORES          # 4 images per core
N_IMG = B_LOC * C             # 16 (b,c) planes per core
HP, WP = H + 2, W + 2         # 514, 514
NBLK = 4                      # row blocks of 128 per plane
P = 128
CSPLIT = 440                  # out cols [0,CSPLIT) via DVE FIR, rest via ACT
SW = W - CSPLIT

_PROGRAM_CACHE = {}


def _register_absdiff():
    """Custom DVE op: out = |in0 - s0| (for the seam-fixup tiles, where the
    channel varies per partition so s0 must be a per-partition scalar)."""
    from concourse import dve_ops
    from concourse.dve_spec import Spec, Src0, C0, relu, lower
    from concourse.dve_uop import DveOpSpec

    for o in dve_ops.OPS:
        if o.name == "ABS_DIFF_ANT":
            return o

    def _ref(in0, in1, s0, s1, imm2):
        s = np.asarray(s0)
        if s.ndim and in0.ndim > s.ndim:
            s = s.reshape(s.shape[0], *([1] * (in0.ndim - 1)))
        return np.abs(in0.astype(np.float32) - s).astype(np.float32)

    spec = Spec(body=relu(Src0 - C0) + relu(C0 - Src0), reference=_ref)
    shas = {
        ver: DveOpSpec(name="ABS_DIFF_ANT", uops=lower(spec, ver=ver)).sha(ver)
        for ver in ("v3", "v4")
    }
    op = dve_ops.DveOp("ABS_DIFF_ANT", spec, subdim=False, uops_sha=shas)
    dve_ops.OPS.append(op)
    dve_ops.CUSTOM_DVE_SPECS[op.name] = spec
    dve_ops._SUB_OPCODE_FOR_NAME[op.name] = (
        dve_ops._CUSTOM_DVE_ROW_BASE + len(dve_ops.OPS) - 1
    )
    return op


def _fir_ref(in0, in1, s0, s1, imm2):
    x = np.asarray(in0, np.float32)
    x1 = np.roll(x, 1, axis=-1)
    x2 = np.roll(x, 2, axis=-1)
    x1[..., 0] = 0
    x2[..., :2] = 0
    return (
        np.abs(x - np.float32(s0))
        + np.abs(x1 - np.float32(s1))
        + np.abs(x2 - np.float32(imm2))
    ).astype(np.float32)


def _register_fir3():
    """Hand-written DVE uop program (bypasses lower()):
    out[n] = |x[n]-s0| + |x[n-1]-s1| + |x[n-2]-imm2|.
    Swap flops at blocks 0/1 act as a 2-deep delay line (BYPASS outputs A
    and latches B; CURR_SWAP_OUT reads last cycle's latch), then three
    ABSOLUTE_DIFF ALU stages + two ADDs. Validated on HW (exp_fir.py)."""
    from concourse import dve_ops
    from concourse.dve_spec import Spec, Src0, C0, C1, C2
    from concourse.dve_uop import (
        DveOpSpec,
        UopConfig,
        AluOp,
        AluInp,
        InpSel,
        DelayInp,
        OutSel,
        OutPath,
        Trigger,
        ENABLE,
    )

    for o in dve_ops.OPS:
        if o.name == "FIR3_ANT":
            return o

    u = UopConfig()
    u.enable_input(InpSel.SRC_0, 1)      # chain0 @B0 = x[n]
    u.enable_input(InpSel.CONST_0, 2)    # chain1 = c0
    u.enable_input(InpSel.CONST_1, 3)    # chain2 = c1
    u.enable_input(InpSel.CONST_2, 4)    # chain3 = c2
    u.require_inp0 = ENABLE
    u.trigger = (Trigger.SRC_TENSOR_DONE, Trigger.NONE, Trigger.NONE)
    u.next_uop = (0, 0, 0)
    u.enable_output(OutSel.ALU_OUT, OutPath.WR0_LO)
    dp = u.datapath_config
    dp[0].enable_alu(AluOp.BYPASS, AluInp.CURR_SWAP_OUT, AluInp.PREV_DELAY_0)
    dp[0].swap_enable = ENABLE           # out = x[n-1], latch x[n]
    dp[0].pass_through_delay(0, 1, 2, 3)
    dp[1].enable_alu(AluOp.BYPASS, AluInp.CURR_SWAP_OUT, AluInp.PREV_ALU_OUT)
    dp[1].swap_enable = ENABLE           # out = x[n-2], latch x[n-1]
    dp[1].pass_through_delay(0, 1, 2, 3)
    dp[1].enable_delay_from_src(DelayInp.PREV_ALU_OUT, 4)  # lane4 = x[n-1]
    dp[2].enable_alu(AluOp.ABSOLUTE_DIFF, AluInp.PREV_ALU_OUT, AluInp.PREV_DELAY_3)
    dp[2].pass_through_delay(0, 1, 2, 4)  # t2 = |x[n-2]-c2|
    dp[3].enable_alu(AluOp.ABSOLUTE_DIFF, AluInp.PREV_DELAY_4, AluInp.PREV_DELAY_2)
    dp[3].pass_through_delay(0, 1)        # t1 = |x[n-1]-c1|
    dp[3].enable_delay_from_src(DelayInp.PREV_ALU_OUT, 5)  # lane5 = t2
    dp[4].enable_alu(AluOp.ADD, AluInp.PREV_ALU_OUT, AluInp.PREV_DELAY_5)
    dp[4].pass_through_delay(0, 1)        # t12 = t1 + t2
    dp[5].enable_alu(AluOp.ABSOLUTE_DIFF, AluInp.PREV_DELAY_0, AluInp.PREV_DELAY_1)
    dp[5].enable_delay_from_src(DelayInp.PREV_ALU_OUT, 2)  # t0; lane2 = t12
    dp[6].enable_alu(AluOp.ADD, AluInp.PREV_ALU_OUT, AluInp.PREV_DELAY_2)
    dp[7].pass_through_alu()
    u.validate("v3")

    row = dve_ops._CUSTOM_DVE_ROW_BASE + len(dve_ops.OPS)
    raw = DveOpSpec(name="FIR3_ANT", uops=[u], opcode=row, rd1_en=False)
    raw.validate("v3")
    spec = Spec(body=(Src0 - C0) + C1 + C2, reference=_fir_ref)
    op = dve_ops.DveOp(
        "FIR3_ANT",
        spec,
        subdim=False,
        uops_sha={v: raw.sha(v) for v in ("v3", "v4")},
    )
    dve_ops.OPS.append(op)
    dve_ops.CUSTOM_DVE_SPECS[op.name] = spec
    dve_ops._SUB_OPCODE_FOR_NAME[op.name] = row
    for ver in ("v3", "v4"):
        dve_ops._COMPILE_CACHE[("FIR3_ANT", ver)] = raw
    return op


# tap t = 3*i + j ; fixup taps computed on ScalarE (rest on VectorE absdiff)
ACT_TAPS = (0, 2, 4, 6, 8)


def _dedup_ldweights(m):
    """bass splits every matmul into Ldweights + Matmult(ldweights=False).
    With consecutive matmuls sharing one stationary, the repeated Ldweights
    serialize PE fill/drain (~384ns/MM measured for N=512 vs ~216 streaming;
    walrus --enable-ldw-opt rejects concourse-emitted LDWs outright). Drop
    any Ldweights whose weights AP equals the previous one's, unless it
    carries semaphore waits. HW-validated (exact result, tiny repro)."""
    removed = 0
    for fn in m.functions:
        for blk in fn.blocks:
            insts = list(blk.instructions)
            keep = []
            last_key = None
            for inst in insts:
                if inst.opcode == "Ldweights":
                    key = repr(inst.ins[0])
                    has_wait = (
                        inst.sync_info is not None
                        and len(inst.sync_info.on_wait) > 0
                    )
                    if key == last_key and not has_wait:
                        removed += 1
                        continue
                    last_key = key
                keep.append(inst)
            if len(keep) != len(insts):
                blk.instructions = keep
    return removed


def _build_program(w9):
    """w9: [C, 9] float32 weight taps (baked into FIR immediates)."""
    import concourse.mybir as mybir
    import concourse.tile as tile
    from concourse import bacc

    f32 = mybir.dt.float32
    bf16 = mybir.dt.bfloat16
    absdiff = _register_absdiff()
    fir3 = _register_fir3()
    nc = bacc.Bacc("TRN2", target_bir_lowering=False)

    xpad = nc.declare_dram_parameter("xpad", [N_IMG, HP, WP], bf16, isOutput=False)
    smat = nc.declare_dram_parameter("smat", [3, P, P], bf16, isOutput=False)
    bias = nc.declare_dram_parameter("bias", [P, (N_IMG + 2) * 18], f32, isOutput=False)
    outp = nc.declare_dram_parameter("outp", [N_IMG, HP, W], f32, isOutput=True)
    # seam rows land here as one [128, W] dump per fixup tile (64 tiny
    # row-pair DMAs serialized ~40us on the Sync queue; host scatters instead)
    outf = nc.declare_dram_parameter("outf", [2, P, W], f32, isOutput=True)

    with tile.TileContext(nc) as tc:
        with (
            tc.tile_pool(name="const", bufs=1) as cpool,
            tc.tile_pool(name="xp", bufs=3) as xpool,
            tc.tile_pool(name="fp", bufs=6) as fpool,
            tc.tile_pool(name="dp", bufs=18) as dpool,
            tc.tile_pool(name="op", bufs=4) as opool,
            tc.tile_pool(name="ps", bufs=2, space="PSUM") as ppool,
        ):
            s_t = cpool.tile([P, 3, P], bf16, tag="s")
            nc.sync.dma_start(out=s_t, in_=smat[:].rearrange("s k p -> k s p"))
            b_all = cpool.tile([P, (N_IMG + 2) * 18], f32, tag="ball")
            nc.sync.dma_start(out=b_all, in_=bias[:])

            # Warmup: attach the ACT table-load pseudo-instruction here.
            warm = cpool.tile([P, 2], f32, tag="warm")
            nc.vector.memset(warm, 0.0)
            nc.scalar.activation(
                out=warm[:, 0:1],
                in_=warm[:, 1:2],
                func=mybir.ActivationFunctionType.Abs,
                bias=0.0,
                scale=1.0,
            )
            nc.scalar.activation(
                out=warm[:, 1:2],
                in_=warm[:, 0:1],
                func=mybir.ActivationFunctionType.Copy,
                scale=-1.0,
            )

            def _evac(ps, st):
                o_t = opool.tile([P, NBLK, W], f32, tag="o")
                nc.scalar.activation(
                    out=o_t,
                    in_=ps,
                    func=mybir.ActivationFunctionType.Copy,
                    scale=1.0,
                )
                nc.sync.dma_start(
                    out=outp[st, 0:512, :].rearrange("(b q) w -> q b w", q=P)[1:127],
                    in_=o_t[1:127],
                )

            def _fixup(fi):
                g0 = fi * 8
                xf = xpool.tile([P, WP], bf16, tag="xf")
                for band in range(4):
                    nc.sync.dma_start(
                        out=xf[32 * band : 32 * (band + 1)],
                        in_=xpad[g0 : g0 + 8, 126 + 128 * band : 130 + 128 * band, :],
                    )
                bofs = (N_IMG + fi) * 18
                df_tiles = []
                for t in range(9):
                    i, j = divmod(t, 3)
                    d = dpool.tile([P, W], bf16, tag="df")
                    src = xf[:, j : j + W]
                    if t in ACT_TAPS:
                        nc.scalar.activation(
                            out=d,
                            in_=src,
                            func=mybir.ActivationFunctionType.Abs,
                            bias=b_all[:, bofs + 9 + t : bofs + 10 + t],
                            scale=1.0,
                        )
                    else:
                        nc.vector._custom_dve(
                            absdiff,
                            out=d,
                            in0=src,
                            s0=b_all[:, bofs + t : bofs + t + 1],
                        )
                    df_tiles.append(d)

                pf = ppool.tile([P, W], mybir.dt.float32, tag="ps")
                for i in range(3):
                    t0 = 3 * i
                    for j in range(3):
                        nc.tensor.matmul(
                            pf,
                            lhsT=s_t[:, i, :],
                            rhs=df_tiles[t0 + j],
                            start=(t0 + j == 0),
                            stop=(t0 + j == 8),
                        )

                of = opool.tile([P, W], f32, tag="o")
                nc.scalar.activation(
                    out=of,
                    in_=pf,
                    func=mybir.ActivationFunctionType.Copy,
                    scale=1.0,
                )
                nc.sync.dma_start(out=outf[fi], in_=of)

            pending = None
            # ---- main supertiles: one per (b,c) plane ----
            for st in range(N_IMG):
                c = st % C
                x_t = xpool.tile([P, NBLK, WP], bf16, tag="x")
                nc.sync.dma_start(
                    out=x_t,
                    in_=xpad[st, 0:512, :].rearrange("(b q) w -> q b w", q=P),
                )
                # DVE: 3 FIR passes (all 3 j-taps of kernel row i fused)
                p_tiles = []
                for i in range(3):
                    pt = fpool.tile([P, NBLK, CSPLIT + 2], bf16, tag="f")
                    nc.vector._custom_dve(
                        fir3,
                        out=pt,
                        in0=x_t[:, :, 0 : CSPLIT + 2],
                        s0=float(w9[c, 3 * i + 2]),
                        s1=float(w9[c, 3 * i + 1]),
                        imm2=float(w9[c, 3 * i + 0]),
                    )
                    p_tiles.append(pt)
                # ACT: single-tap planes for the last SW output columns
                bofs = st * 18
                d_tiles = []
                for t in range(9):
                    i, j = divmod(t, 3)
                    d = dpool.tile([P, NBLK, SW], bf16, tag="d")
                    nc.scalar.activation(
                        out=d,
                        in_=x_t[:, :, CSPLIT + j : CSPLIT + j + SW],
                        func=mybir.ActivationFunctionType.Abs,
                        bias=b_all[:, bofs + 9 + t : bofs + 10 + t],
                        scale=1.0,
                    )
                    d_tiles.append(d)

                # evacuate the PREVIOUS supertile's psum now, after this
                # supertile's engine work was queued, so ACT stays busy and
                # PE never starves at the supertile boundary.
                if pending is not None:
                    _evac(*pending)
                    pending = None

                ps = ppool.tile([P, NBLK, W], mybir.dt.float32, tag="ps")
                for i in range(3):
                    for blk in range(NBLK):
                        nc.tensor.matmul(
                            ps[:, blk, 0:CSPLIT],
                            lhsT=s_t[:, i, :],
                            rhs=p_tiles[i][:, blk, 2 : CSPLIT + 2],
                            start=(i == 0),
                            stop=False,
                        )
                        for j in range(3):
                            nc.tensor.matmul(
                                ps[:, blk, CSPLIT:W],
                                lhsT=s_t[:, i, :],
                                rhs=d_tiles[3 * i + j][:, blk, :],
                                start=False,
                                stop=(i == 2 and j == 2),
                            )
                pending = (ps, st)

            if pending is not None:
                _evac(*pending)
                pending = None
            for fi in range(2):
                _fixup(fi)
    _dedup_ldweights(nc.m)
    nc.finalize()
    return nc


def _get_program(w9):
    key = w9.tobytes()
    if _PROGRAM_CACHE.get("key") != key:
        _PROGRAM_CACHE["nc"] = _build_program(w9)
        _PROGRAM_CACHE["key"] = key
    return _PROGRAM_CACHE["nc"]


def _host_consts(weight):
    """Negated shift matrices + per-partition bias tables (shared by cores)."""
    w9 = np.asarray(weight, np.float32).reshape(C, 9)  # [c, t]

    S = np.zeros((3, P, P), np.float32)
    for i in range(3):
        for p in range(P):
            k = p + i - 1
            if 0 <= k < P:
                S[i, k, p] = -1.0
    S = S.astype(ml_dtypes.bfloat16)

    # bias table [P, (N_IMG+2)*18]: cols st*18+t = +w (DVE absdiff taps),
    # st*18+9+t = -w (ACT taps); supertiles 0-15 then fixup tiles 16-17
    bias = np.zeros((P, (N_IMG + 2) * 18), np.float32)
    for st in range(N_IMG):
        c = st % C
        bias[:, st * 18 : st * 18 + 9] = w9[c][None, :]
        bias[:, st * 18 + 9 : st * 18 + 18] = -w9[c][None, :]
    for fi in range(2):  # fixup tiles: partition q = 32*band + 4*g + r
        o = (N_IMG + fi) * 18
        for band in range(4):
            for g in range(8):
                c = (fi * 8 + g) % C
                lo = 32 * band + 4 * g
                bias[lo : lo + 4, o : o + 9] = w9[c][None, :]
                bias[lo : lo + 4, o + 9 : o + 18] = -w9[c][None, :]
    return S, bias, w9


def kernel(input, weight):
    from concourse.bass_utils import run_bass_kernel_spmd

    x = np.asarray(input, np.float32)
    S, bias, w9 = _host_consts(weight)

    xpad = np.pad(x, ((0, 0), (0, 0), (1, 1), (1, 1))).astype(ml_dtypes.bfloat16)
    in_maps = []
    for core in range(N_CORES):
        shard = np.ascontiguousarray(
            xpad[core * B_LOC : (core + 1) * B_LOC].reshape(N_IMG, HP, WP)
        )
        in_maps.append({"xpad": shard, "smat": S, "bias": bias})

    nc = _get_program(w9)
    res = run_bass_kernel_spmd(nc, in_maps, core_ids=list(range(N_CORES)))

    out = np.empty((B, C, H, W), np.float32)
    for core in range(N_CORES):
        o = np.array(res.results[core]["outp"]).reshape(N_IMG, HP, W)
        # scatter the seam rows from the fixup dumps (partition layout
        # q = 32*band + 4*g + r; valid rows r=1,2 -> padded rows 127+128b+{0,1})
        ofx = res.results[core]["outf"]  # [2, 128, W]
        for fi in range(2):
            g0 = fi * 8
            blk = ofx[fi].reshape(4, 8, 4, W)  # [band, g, r, W]
            for band in range(4):
                o[g0 : g0 + 8, 127 + 128 * band : 129 + 128 * band, :] = blk[
                    band, :, 1:3, :
                ]
        o = o.reshape(B_LOC, C, HP, W)
        out[core * B_LOC : (core + 1) * B_LOC] = o[:, :, 1 : H + 1, :]
    return out
